# revision 1
# baseline (speedup 1.0000x reference)
"""Trainium2 Bass kernel for nn_AgentPredictor.

Reference computation per batch element b (B = 1048576, N = 16 others, D = 8, H = 16):
    enc(x)    = relu(x @ W_enc + b_enc)            x in R^2 -> R^8
    focal_emb = enc(focal)                         (8,)
    others_emb= enc(others[n]) for n in 0..15      (16, 8)
    query     = focal_emb @ W_q + b_q              (8,)
    scores_n  = <others_emb[n], query> / sqrt(8)   (16,)
    weights   = softmax(scores)                    (16,)
    ctx       = sum_n weights_n * others_emb[n]    (8,)
    dec_in    = [fruit, focal, ctx]                (11,)
    h         = relu(dec_in @ W_d1 + b_d1)         (16,)
    y         = sigmoid(h @ W_d2 + b_d2)           (1,)

Strategy: pure data parallelism over 8 cores.  Within a core, "p-major" flat
layout: core batch (131072) -> [128 partitions, 1024 columns], batch index =
p * 1024 + t; all DMAs are full-width contiguous transfers.  Compute is
elementwise/tree-reduce in natural layout (batch on partitions) in fp16
(~2e-4 rel error), pipelined in chunks of DC element columns across
DVE / GPSIMD / ACT.  Every large DVE op keeps inner-stride-1 access
patterns on all operands so the DVE's fp16 2x mode engages: the encoder
runs in (t, d, n) free layout (d-broadcasts land on the middle dim), the
attention-score stage in (t, n, d); the encoder output is materialized in
both layouts by two ACT relu passes.
"""

import sys

if "/opt/trn_rl_repo" not in sys.path:
    sys.path.insert(0, "/opt/trn_rl_repo")

import numpy as np

import concourse.bass as bass
import concourse.mybir as mybir
import concourse.tile as tile
from concourse import bass_utils

# Problem sizes (hardcoded per contract)
B = 1048576
N_CORES = 8
BC = B // N_CORES          # 131072 per core
P = 128
A = BC // P                # 1024 element-columns per partition
N = 16                     # other agents
D = 8                      # embedding dim
H = 16                     # decision hidden
DEC = 3 + D                # 11 decision inputs
SCALE = 1.0 / np.sqrt(D)

F32 = mybir.dt.float32
F16 = mybir.dt.float16

# Tiling parameter: element-columns per pipeline chunk
DC = 32

# wconsts column layout (f32 source; an fp16 staged copy is made on device)
W0DN_OFF = 0               # [d*16+n] -> W_enc[0, d]     (128)  (d,n) layout
W1DN_OFF = 128             # [d*16+n] -> W_enc[1, d]     (128)
BEDN_OFF = 256             # [d*16+n] -> b_enc[d]        (128)
WQ_OFF = 384               # [j*8+i] -> W_q[i, j]        (64)
BQ_OFF = 448               # [j]     -> b_q[j]           (8)
W1D_OFF = 456              # [j*11+i] -> W_d1[i, j]      (176)
B1_OFF = 632               # [j]     -> b_d1[j]          (16)
W2D_OFF = 648              # [j]     -> W_d2[j, 0]       (16)
B2_OFF = 664               # b_d2[0]                     (1)
W0F_OFF = 665              # [i]     -> W_enc[0, 0:8]    (8)
W1F_OFF = 673              # [i]     -> W_enc[1, 0:8]    (8)
BEF_OFF = 681              # [i]     -> b_enc[0:8]       (8)
WC_COLS = 690


def _build_wconsts(W_enc, b_enc, W_q, b_q, W_d1, b_d1, W_d2, b_d2):
    wc = np.zeros((WC_COLS,), dtype=np.float32)
    wc[W0DN_OFF:W0DN_OFF + 128] = np.repeat(W_enc[0, :], N)
    wc[W1DN_OFF:W1DN_OFF + 128] = np.repeat(W_enc[1, :], N)
    wc[BEDN_OFF:BEDN_OFF + 128] = np.repeat(b_enc, N)
    wc[WQ_OFF:WQ_OFF + 64] = W_q.T.reshape(-1)          # [j, i] row-major
    wc[BQ_OFF:BQ_OFF + 8] = b_q
    wc[W1D_OFF:W1D_OFF + 176] = W_d1.T.reshape(-1)      # [j, i] row-major
    wc[B1_OFF:B1_OFF + 16] = b_d1
    wc[W2D_OFF:W2D_OFF + 16] = W_d2[:, 0]
    wc[B2_OFF] = b_d2[0]
    wc[W0F_OFF:W0F_OFF + 8] = W_enc[0, :]
    wc[W1F_OFF:W1F_OFF + 8] = W_enc[1, :]
    wc[BEF_OFF:BEF_OFF + 8] = b_enc
    return np.broadcast_to(wc, (P, WC_COLS)).copy()


def _hoist_multi_waits(nc):
    """Compute instructions support a single sync-wait slot.  Where the Tile
    scheduler emitted more than one wait, hoist the extra waits onto Drain
    instructions inserted just before (one wait per Drain), leaving the
    compute instruction with a single wait.  Semantics are identical: the
    engine executes the Drains' waits in order, then the instruction."""
    n_fixed = 0
    for f in nc.m.functions:
        for blk in f.blocks:
            ins = blk.instructions
            i = 0
            while i < len(ins):
                inst = ins[i]
                op = str(inst.opcode)
                if op == "EventSemaphore":
                    i += 1
                    continue
                si = inst.sync_info
                waits = list(si.on_wait) if si is not None else []
                if len(waits) > 1:
                    for k, w in enumerate(waits[:-1]):
                        d = mybir.InstDrain(
                            name=f"W{k}-{inst.name}", ins=[], outs=[]
                        )
                        d.engine = inst.engine
                        d.sync_info = mybir.SyncInfo(on_wait=[w], on_update=[])
                        ins.insert(i, d)
                        i += 1
                    inst.sync_info = mybir.SyncInfo(
                        on_wait=[waits[-1]], on_update=list(si.on_update)
                    )
                    n_fixed += 1
                i += 1
    return n_fixed


def build_nc():
    nc = bass.Bass()

    fruit = nc.dram_tensor("fruit", [P, A], F32, kind="ExternalInput")
    focal = nc.dram_tensor("focal", [P, 2 * A], F32, kind="ExternalInput")
    others = nc.dram_tensor("others", [P, 32 * A], F32, kind="ExternalInput")
    wcd = nc.dram_tensor("wconsts", [P, WC_COLS], F32, kind="ExternalInput")
    out = nc.dram_tensor("out", [P, A], F32, kind="ExternalOutput")

    AF = mybir.ActivationFunctionType
    ALU = mybir.AluOpType
    AX = mybir.AxisListType

    with tile.TileContext(nc) as tc:
        with tc.tile_pool(name="persist", bufs=1) as persist:
            # --- persistent loads ------------------------------------------------
            wc_dma = persist.tile([P, WC_COLS], F32)
            nc.sync.dma_start(out=wc_dma, in_=wcd[:, :])
            wc = persist.tile([P, WC_COLS], F32)
            nc.vector.tensor_copy(out=wc, in_=wc_dma)
            wc16 = persist.tile([P, WC_COLS], F16)
            nc.vector.tensor_copy(out=wc16, in_=wc_dma)
            fruit_s = persist.tile([P, A], F32)
            nc.sync.dma_start(out=fruit_s, in_=fruit[:, :])
            focal_s = persist.tile([P, 2 * A], F32)
            nc.sync.dma_start(out=focal_s, in_=focal[:, :])
            out_s = persist.tile([P, A], F32)

            # full-width encoder bias, materialized once for accum-DMAs
            beb_full = persist.tile([P, DC, D, N], F16)

            # const views ((d, n) layout for the encoder)
            w0dn = wc16[:, W0DN_OFF:W0DN_OFF + 128].rearrange(
                "p (d n) -> p d n", n=N
            )
            w1dn = wc16[:, W1DN_OFF:W1DN_OFF + 128].rearrange(
                "p (d n) -> p d n", n=N
            )
            bedn = wc16[:, BEDN_OFF:BEDN_OFF + 128]
            w0f16 = wc16[:, W0F_OFF:W0F_OFF + 8]
            w1f16 = wc16[:, W1F_OFF:W1F_OFF + 8]
            bef16 = wc16[:, BEF_OFF:BEF_OFF + 8]
            wq16 = wc16[:, WQ_OFF:WQ_OFF + 64].rearrange("p (j i) -> p j i", i=D)
            bq = wc[:, BQ_OFF:BQ_OFF + 8]
            w1d16 = wc16[:, W1D_OFF:W1D_OFF + 176].rearrange(
                "p (j i) -> p j i", i=DEC
            )
            b1 = wc[:, B1_OFF:B1_OFF + 16]
            w2d = wc[:, W2D_OFF:W2D_OFF + 16]
            b2 = wc[:, B2_OFF:B2_OFF + 1]

            nc.vector.tensor_copy(
                out=beb_full,
                in_=wc16[:, BEDN_OFF:BEDN_OFF + 128].rearrange(
                    "p (d n) -> p d n", n=N
                ).unsqueeze(1).broadcast_to([P, DC, D, N]),
            )

            # ---- focal encode + query for ALL columns up front --------------
            # The query chain only depends on `focal`, so it runs once in a
            # pre-loop with 4x-bigger ops (quarter the per-op overhead) and a
            # scratch pool that is freed before the main loop starts.
            q16_all = persist.tile([P, A, D], F16)
            DCQ = 256
            enc_pools = (
                tc.tile_pool(name="oin", bufs=2),
                tc.tile_pool(name="enc3", bufs=3),
                tc.tile_pool(name="encm", bufs=2),
            )
            oin_pool = enc_pools[0].__enter__()
            enc3_pool = enc_pools[1].__enter__()
            encm_pool = enc_pools[2].__enter__()
            with tc.tile_pool(name="preq", bufs=2) as preq:
                for qc in range(A // DCQ):
                    qc0 = qc * DCQ
                    foc = focal_s[:, 2 * qc0:2 * (qc0 + DCQ)].rearrange(
                        "p (t k) -> p t k", k=2
                    )
                    foc16 = preq.tile([P, DCQ, 2], F16, tag="qfoc")
                    nc.scalar.copy(out=foc16, in_=foc)
                    f0b = foc16[:, :, 0].unsqueeze(2).broadcast_to([P, DCQ, D])
                    f1b = foc16[:, :, 1].unsqueeze(2).broadcast_to([P, DCQ, D])
                    w0fb = w0f16.unsqueeze(1).broadcast_to([P, DCQ, D])
                    w1fb = w1f16.unsqueeze(1).broadcast_to([P, DCQ, D])

                    fm0 = preq.tile([P, DCQ, D], F16, tag="qfm0")
                    nc.vector.tensor_tensor(out=fm0, in0=f0b, in1=w0fb,
                                            op=ALU.mult)
                    fm1 = preq.tile([P, DCQ, D], F16, tag="qfm1")
                    nc.vector.tensor_tensor(out=fm1, in0=f1b, in1=w1fb,
                                            op=ALU.mult)
                    nc.vector.tensor_tensor(out=fm0, in0=fm0, in1=fm1,
                                            op=ALU.add)
                    befb = bef16.unsqueeze(1).broadcast_to([P, DCQ, D])
                    nc.vector.tensor_tensor(out=fm0, in0=fm0, in1=befb,
                                            op=ALU.add)
                    ef = preq.tile([P, DCQ, D], F16, tag="qef")
                    nc.vector.tensor_scalar_max(ef, fm0, 0.0)

                    efb = ef.unsqueeze(2).broadcast_to([P, DCQ, D, D])
                    wqb = wq16.unsqueeze(1).broadcast_to([P, DCQ, D, D])
                    qm = preq.tile([P, DCQ, D, D], F16, tag="qqm")
                    nc.vector.tensor_tensor(out=qm, in0=efb, in1=wqb,
                                            op=ALU.mult)
                    nc.vector.tensor_tensor(
                        out=qm[:, :, :, 0:4], in0=qm[:, :, :, 0:4],
                        in1=qm[:, :, :, 4:8], op=ALU.add,
                    )
                    nc.vector.tensor_tensor(
                        out=qm[:, :, :, 0:2], in0=qm[:, :, :, 0:2],
                        in1=qm[:, :, :, 2:4], op=ALU.add,
                    )
                    qf = preq.tile([P, DCQ, D], F32, tag="qqf")
                    nc.vector.tensor_tensor(
                        out=qf, in0=qm[:, :, :, 0], in1=qm[:, :, :, 1],
                        op=ALU.add,
                    )
                    bqb = bq.unsqueeze(1).broadcast_to([P, DCQ, D])
                    nc.vector.tensor_tensor(
                        out=q16_all[:, qc0:qc0 + DCQ, :], in0=qf, in1=bqb,
                        op=ALU.add,
                    )

            main_pools = (
                tc.tile_pool(name="dcn", bufs=2),
            )
            dcn_pool = main_pools[0].__enter__()

            for dc in range(A // DC):
                c0 = dc * DC
                # others chunk: [128, DC*32] contiguous f32
                o_in = oin_pool.tile([P, DC, N, 2], F32, tag="o_in")
                nc.sync.dma_start(
                    out=o_in, in_=others[:, c0 * 32:(c0 + DC) * 32]
                )
                # split fp16 casts of the two per-agent features (ACT)
                o1 = enc3_pool.tile([P, DC, N], F16, tag="o1")
                nc.scalar.copy(out=o1, in_=o_in[:, :, :, 1])
                o0 = enc3_pool.tile([P, DC, N], F16, tag="o0")
                nc.scalar.copy(out=o0, in_=o_in[:, :, :, 0])

                foc = focal_s[:, 2 * c0:2 * (c0 + DC)].rearrange(
                    "p (t k) -> p t k", k=2
                )
                q16 = q16_all[:, c0:c0 + DC, :]

                # dec-input assembly (fp16)
                dec = dcn_pool.tile([P, DC, DEC], F16, tag="dec")
                nc.scalar.copy(
                    out=dec[:, :, 0:1],
                    in_=fruit_s[:, c0:c0 + DC].unsqueeze(2),
                )
                nc.scalar.copy(out=dec[:, :, 1:3], in_=foc)

                # ---- encoder (others), (t, d, n) layout ---------------------
                o0b = o0.unsqueeze(2).broadcast_to([P, DC, D, N])
                o1b = o1.unsqueeze(2).broadcast_to([P, DC, D, N])
                w0b = w0dn.unsqueeze(1).broadcast_to([P, DC, D, N])
                w1b = w1dn.unsqueeze(1).broadcast_to([P, DC, D, N])
                beb = bedn.rearrange("p (d n) -> p d n", n=N).unsqueeze(
                    1
                ).broadcast_to([P, DC, D, N])

                m0 = encm_pool.tile([P, DC, D, N], F16, tag="m0")
                nc.vector.tensor_tensor(out=m0, in0=o0b, in1=w0b, op=ALU.mult)
                m1 = encm_pool.tile([P, DC, D, N], F16, tag="m1")
                nc.gpsimd.tensor_tensor(out=m1, in0=o1b, in1=w1b, op=ALU.mult)
                # The two encoder adds run on the DMA engines' inline CCE
                # adders, explicitly sliced into half-chunk DMAs so every
                # per-partition descriptor run stays within the CCE's max
                # element count (2048); full-size runs fail at runtime.
                half = DC // 2
                nc.gpsimd.dma_start(
                    out=m0[:, 0:half], in_=m1[:, 0:half], accum_op=ALU.add
                )
                nc.gpsimd.dma_start(
                    out=m0[:, half:DC], in_=m1[:, half:DC], accum_op=ALU.add
                )
                nc.gpsimd.dma_start(
                    out=m0[:, 0:half], in_=beb_full[:, 0:half], accum_op=ALU.add
                )
                nc.gpsimd.dma_start(
                    out=m0[:, half:DC], in_=beb_full[:, half:DC],
                    accum_op=ALU.add,
                )
                # relu into both layouts (ACT)
                eo_tdn = dcn_pool.tile([P, DC, D, N], F16, tag="eo_tdn")
                nc.scalar.activation(out=eo_tdn, in_=m0, func=AF.Relu)
                eo_tnd = dcn_pool.tile([P, DC, N, D], F16, tag="eo_tnd")
                nc.scalar.activation(
                    out=eo_tnd, in_=m0[:].transpose([0, 1, 3, 2]), func=AF.Relu
                )

                # ---- scores in (t, n, d): fp16 mul + tree over d ------------
                qsub = q16.unsqueeze(2).broadcast_to([P, DC, N, D])
                scp = dcn_pool.tile([P, DC, N, D], F16, tag="scp")
                nc.vector.tensor_tensor(out=scp, in0=eo_tnd, in1=qsub, op=ALU.mult)
                nc.vector.tensor_tensor(
                    out=scp[:, :, :, 0:4], in0=scp[:, :, :, 0:4],
                    in1=scp[:, :, :, 4:8], op=ALU.add,
                )
                nc.vector.tensor_tensor(
                    out=scp[:, :, :, 0:2], in0=scp[:, :, :, 0:2],
                    in1=scp[:, :, :, 2:4], op=ALU.add,
                )
                scores = dcn_pool.tile([P, DC, N], F32, tag="scores")
                nc.vector.tensor_tensor(
                    out=scores, in0=scp[:, :, :, 0], in1=scp[:, :, :, 1],
                    op=ALU.add,
                )

                # ---- softmax over n ----------------------------------------
                e = dcn_pool.tile([P, DC, N], F32, tag="e")
                nc.scalar.activation(out=e, in_=scores, func=AF.Exp, scale=SCALE)
                ssum = dcn_pool.tile([P, DC], F32, tag="ssum")
                nc.vector.tensor_reduce(out=ssum, in_=e, axis=AX.X, op=ALU.add)
                rinv = dcn_pool.tile([P, DC], F32, tag="rinv")
                nc.vector.reciprocal(out=rinv, in_=ssum)
                w16 = dcn_pool.tile([P, DC, N], F16, tag="w16")
                nc.vector.tensor_tensor(
                    out=w16,
                    in0=e,
                    in1=rinv.unsqueeze(2).broadcast_to([P, DC, N]),
                    op=ALU.mult,
                )

                # ---- context in (t, d, n): fp16 mul + tree over n -> dec ----
                wb = w16.unsqueeze(2).broadcast_to([P, DC, D, N])
                cxp = dcn_pool.tile([P, DC, D, N], F16, tag="cxp")
                nc.vector.tensor_tensor(out=cxp, in0=eo_tdn, in1=wb, op=ALU.mult)
                nc.vector.tensor_tensor(
                    out=cxp[:, :, :, 0:8], in0=cxp[:, :, :, 0:8],
                    in1=cxp[:, :, :, 8:16], op=ALU.add,
                )
                nc.vector.tensor_tensor(
                    out=cxp[:, :, :, 0:4], in0=cxp[:, :, :, 0:4],
                    in1=cxp[:, :, :, 4:8], op=ALU.add,
                )
                nc.vector.tensor_tensor(
                    out=cxp[:, :, :, 0:2], in0=cxp[:, :, :, 0:2],
                    in1=cxp[:, :, :, 2:4], op=ALU.add,
                )
                nc.vector.tensor_tensor(
                    out=dec[:, :, 3:11],
                    in0=cxp[:, :, :, 0], in1=cxp[:, :, :, 1], op=ALU.add,
                )

                # ---- decision net: fp16 mul + tree over i=11 ----------------
                db = dec.unsqueeze(2).broadcast_to([P, DC, H, DEC])
                w1db = w1d16.unsqueeze(1).broadcast_to([P, DC, H, DEC])
                dm = dcn_pool.tile([P, DC, H, DEC], F16, tag="dm")
                nc.vector.tensor_tensor(out=dm, in0=db, in1=w1db, op=ALU.mult)
                # 11 = 5 + 5 + 1
                nc.vector.tensor_tensor(
                    out=dm[:, :, :, 0:5], in0=dm[:, :, :, 0:5],
                    in1=dm[:, :, :, 5:10], op=ALU.add,
                )
                nc.vector.tensor_tensor(
                    out=dm[:, :, :, 0:2], in0=dm[:, :, :, 0:2],
                    in1=dm[:, :, :, 2:4], op=ALU.add,
                )
                nc.vector.tensor_tensor(
                    out=dm[:, :, :, 0], in0=dm[:, :, :, 0],
                    in1=dm[:, :, :, 1], op=ALU.add,
                )
                nc.vector.tensor_tensor(
                    out=dm[:, :, :, 0], in0=dm[:, :, :, 0],
                    in1=dm[:, :, :, 4], op=ALU.add,
                )
                hp = dcn_pool.tile([P, DC, H], F16, tag="hp")
                nc.vector.tensor_tensor(
                    out=hp, in0=dm[:, :, :, 0], in1=dm[:, :, :, 10], op=ALU.add
                )

                # ---- decision tail -----------------------------------------
                b1b = wc16[:, B1_OFF:B1_OFF + 16].unsqueeze(1).broadcast_to(
                    [P, DC, H]
                )
                nc.vector.tensor_tensor(out=hp, in0=hp, in1=b1b, op=ALU.add)
                h = dcn_pool.tile([P, DC, H], F16, tag="h")
                nc.vector.tensor_scalar_max(h, hp, 0.0)
                w2b = wc16[:, W2D_OFF:W2D_OFF + 16].unsqueeze(1).broadcast_to(
                    [P, DC, H]
                )
                ym = dcn_pool.tile([P, DC, H], F16, tag="ym")
                nc.vector.tensor_tensor(out=ym, in0=h, in1=w2b, op=ALU.mult)
                yp = dcn_pool.tile([P, DC], F32, tag="yp")
                nc.vector.tensor_reduce(out=yp, in_=ym, axis=AX.X, op=ALU.add)
                nc.scalar.activation(
                    out=out_s[:, c0:c0 + DC],
                    in_=yp,
                    func=AF.Sigmoid,
                    bias=b2,
                )
                if dc % 8 == 7:
                    w0c = (dc - 7) * DC
                    nc.sync.dma_start(
                        out=out[:, w0c:c0 + DC], in_=out_s[:, w0c:c0 + DC]
                    )

            for mp in reversed(main_pools):
                mp.__exit__(None, None, None)
            for ep in reversed(enc_pools):
                ep.__exit__(None, None, None)

    _hoist_multi_waits(nc)
    return nc


_NC_CACHE = None


def kernel(fruit_level, focal_features, others_features,
           W_enc, b_enc, W_q, b_q, W_d1, b_d1, W_d2, b_d2):
    global _NC_CACHE
    if _NC_CACHE is None:
        _NC_CACHE = build_nc()
    nc = _NC_CACHE

    wc_np = _build_wconsts(
        np.asarray(W_enc, dtype=np.float32), np.asarray(b_enc, dtype=np.float32),
        np.asarray(W_q, dtype=np.float32), np.asarray(b_q, dtype=np.float32),
        np.asarray(W_d1, dtype=np.float32), np.asarray(b_d1, dtype=np.float32),
        np.asarray(W_d2, dtype=np.float32), np.asarray(b_d2, dtype=np.float32),
    )

    fruit_np = np.ascontiguousarray(np.asarray(fruit_level, dtype=np.float32))
    focal_np = np.ascontiguousarray(np.asarray(focal_features, dtype=np.float32))
    others_np = np.ascontiguousarray(np.asarray(others_features, dtype=np.float32))

    in_maps = []
    for c in range(N_CORES):
        lo, hi = c * BC, (c + 1) * BC
        in_maps.append({
            "fruit": fruit_np[lo:hi].reshape(P, A),
            "focal": focal_np[lo:hi].reshape(P, 2 * A),
            "others": others_np[lo:hi].reshape(P, 32 * A),
            "wconsts": wc_np,
        })

    res = bass_utils.run_bass_kernel_spmd(nc, in_maps, core_ids=list(range(N_CORES)))
    if res.exec_time_ns is not None:
        print(f"HW exec time: {res.exec_time_ns} ns", flush=True)
    outs = [r["out"].reshape(BC, 1) for r in res.results]
    return np.concatenate(outs, axis=0)

